# revision 13
# baseline (speedup 1.0000x reference)
"""Trainium2 Bass kernel for multi-head attention.

Problem: B=4, H=16, S=2048, D=128, fp32.
  scores = (q @ k^T) / sqrt(128); probs = softmax(scores, -1); out = probs @ v

Sharding: 64 (b,h) pairs -> 8 cores x 8 pairs. Fully independent per pair.

Per-(b,h) layout on device (T-layout, contraction dims on partitions):
  qT, kT: [D=128, S=2048] fp16 in SBUF (host pre-transposes + casts).
  For each t-tile (128 keys):
    scoresT[t, s] = kT[:, t-tile].T @ qT      (PE fp16, fp32 psum)
    expT = exp(scoresT / sqrt(D))             (ACT, fused scale, fp16 out)
    outT[d, s]  += v_tile[t, :].T @ expT      (PE fp16, accumulate in psum)
    eacc += expT                              (DVE fp16, 2x perf mode)
  denom = ones.T @ eacc (PE), rec = 1/denom (DVE), out = outT * rec (DVE).

The whole kernel is ONE flat software pipeline over work items
w = (pair, half, t): QK matmuls run 2 items ahead of the exp stream and
cross half/pair boundaries, so the ACT engine (the roofline: S^2/128
lanes @ 1.2 GHz ~= 266 us/core) never starves.  The normalize tail of
each half is chunked in 512-wide halves (oacc lives in 2 single-bank
psum tiles) so the next half's PV accumulation only waits for the chunk
it replaces.

PSUM: scores pool 3 slots x 2 banks (sacc shares the pool slots at each
half's tail) + 2 x 1-bank outT accumulators = 8 banks exactly.

fp16 everywhere off the psum path: q/k/v inputs, exp outputs, denominator
accumulator.  End-to-end L2 error vs the fp32 reference ~3e-4.
"""

import sys

sys.path.insert(0, "/opt/trn_rl_repo")

import numpy as np

B, H, S, D = 4, 16, 2048, 128
N_CORES = 8
BH = B * H                      # 64 pairs
BH_PER_CORE = BH // N_CORES     # 8
T_TILES = S // 128              # 16
S_HALF = S // 2                 # 1024
N_HALVES = BH_PER_CORE * 2      # 16 halves per core
N_ITEMS = N_HALVES * T_TILES    # 256 work items per core
SCALE = float(D) ** -0.5
LOOKAHEAD = 2                   # QK runs this many items ahead of exp

_cache = {}


def _build_program():
    import concourse.tile as tile
    from concourse import bacc, mybir

    F32 = mybir.dt.float32
    F16 = mybir.dt.float16

    nc = bacc.Bacc("TRN2", target_bir_lowering=False, debug=False)

    qt = nc.dram_tensor("qt", [BH_PER_CORE, D, S], F16, kind="ExternalInput")
    kt = nc.dram_tensor("kt", [BH_PER_CORE, D, S], F16, kind="ExternalInput")
    # v pre-shuffled on host to [p, t, d] so the load is fully contiguous
    v = nc.dram_tensor("v", [BH_PER_CORE, 128, T_TILES * D], F16, kind="ExternalInput")
    ot = nc.dram_tensor("ot", [BH_PER_CORE, D, S], F32, kind="ExternalOutput")

    with tile.TileContext(nc) as tc:
        with (
            tc.tile_pool(name="const", bufs=1) as const,
            tc.tile_pool(name="rin", bufs=2) as rin,
            tc.tile_pool(name="exps", bufs=8) as exps,
            tc.tile_pool(name="accp", bufs=3) as accp,
            tc.tile_pool(name="outs", bufs=6) as outs,
            tc.tile_pool(name="psc", bufs=3, space="PSUM") as psc,
            tc.tile_pool(name="pacc", bufs=1, space="PSUM") as pacc,
        ):
            ones_f = const.tile([128, 128], F32)
            nc.vector.memset(ones_f[:], 1.0)
            ones_h = const.tile([128, 128], F16)
            nc.vector.tensor_copy(ones_h[:], ones_f[:])

            # PE warm-up: dummy matmuls while the first input DMAs are in
            # flight, so the tensor engine's p-state is ramped (0.65 -> 2.4
            # GHz) by the time real work arrives.  Output is never read; the
            # first real scores matmul overwrites the slot with start=True.
            warm = psc.tile([128, S_HALF], F32, tag="sc", name="warm")
            for wi in range(8):
                nc.tensor.matmul(
                    warm[:, :128], ones_h[:], ones_h[:], start=True, stop=True
                )

            # per-pair input tiles, managed by the flat pipeline
            rin_tiles = {}   # pair -> (q_r, k_r, v_r)

            def load_pair(i):
                q_r = rin.tile([D, S], F16, tag="q_r")
                k_r = rin.tile([D, S], F16, tag="k_r")
                v_r = rin.tile([128, T_TILES, D], F16, tag="v_r")
                # order: what the first tiles need comes first.  All 16
                # k-tiles and v-tiles are consumed within the FIRST half, so
                # k gets priority over the second half of q.
                v_in = v[i].rearrange("p (t d) -> p t d", t=T_TILES)
                nc.sync.dma_start(out=k_r[:, :128], in_=kt[i, :, :128])
                nc.sync.dma_start(out=q_r[:, :512], in_=qt[i, :, :512])
                nc.sync.dma_start(out=q_r[:, 512:S_HALF], in_=qt[i, :, 512:S_HALF])
                nc.sync.dma_start(out=v_r[:, :2], in_=v_in[:, :2])
                nc.sync.dma_start(out=k_r[:, 128:S_HALF], in_=kt[i, :, 128:S_HALF])
                nc.sync.dma_start(out=k_r[:, S_HALF:], in_=kt[i, :, S_HALF:])
                nc.sync.dma_start(out=v_r[:, 2:], in_=v_in[:, 2:])
                nc.sync.dma_start(out=q_r[:, S_HALF:], in_=qt[i, :, S_HALF:])
                rin_tiles[i] = (q_r, k_r, v_r)

            # per-half state, filled lazily by the pipeline
            half_state = {}  # half index -> dict

            def qk(w):
                """Emit the scores matmuls for work item w (PE)."""
                half, t = divmod(w, T_TILES)
                pair, h = divmod(half, 2)
                if t == 0 and h == 0:
                    load_pair(pair)
                q_r, k_r, _ = rin_tiles[pair]
                s0 = h * S_HALF
                sc = psc.tile([128, S_HALF], F32, tag="sc", name=f"sc_{w}")
                for c in range(0, S_HALF, 512):
                    nc.tensor.matmul(
                        sc[:, c : c + 512],
                        k_r[:, t * 128 : (t + 1) * 128],
                        q_r[:, s0 + c : s0 + c + 512],
                        start=True,
                        stop=True,
                    )
                st = half_state.setdefault(
                    half, {"sc": {}, "ets": {}, "oacc": None, "eacc": None}
                )
                st["sc"][t] = sc

            def exp(w):
                """Emit the exp for work item w (ACT)."""
                half, t = divmod(w, T_TILES)
                st = half_state[half]
                et = exps.tile([128, S_HALF], F16, tag="et", name=f"et_{w}")
                nc.scalar.activation(
                    et[:],
                    st["sc"].pop(t)[:],
                    mybir.ActivationFunctionType.Exp,
                    scale=SCALE,
                )
                st["ets"][t] = et

            def consume(w):
                """Emit PV + denominator accumulation for item w; at the end
                of a half, the fold/recip/mul/store tail (chunked)."""
                half, t = divmod(w, T_TILES)
                pair, h = divmod(half, 2)
                _, _, v_r = rin_tiles[pair]
                st = half_state[half]
                if t == 0:
                    st["oacc"] = [
                        pacc.tile(
                            [128, 512], F32, tag=f"oacc{ci}", name=f"oacc{ci}_{half}"
                        )
                        for ci in range(2)
                    ]
                et = st["ets"][t]
                for ci, c in enumerate(range(0, S_HALF, 512)):
                    nc.tensor.matmul(
                        st["oacc"][ci][:],
                        v_r[:, t, :],
                        et[:, c : c + 512],
                        start=(t == 0),
                        stop=(t == T_TILES - 1),
                    )
                if t == 1:
                    st["eacc"] = accp.tile(
                        [128, S_HALF], F16, tag="eacc", name=f"eacc_{half}"
                    )
                    nc.vector.tensor_add(
                        st["eacc"][:], st["ets"].pop(0)[:], st["ets"].pop(1)[:]
                    )
                elif t > 1:
                    nc.vector.tensor_add(
                        st["eacc"][:], st["eacc"][:], st["ets"].pop(t)[:]
                    )
                if t == T_TILES - 1:
                    s0 = h * S_HALF
                    if half == N_HALVES - 1:
                        # Final half: nothing follows, so psum pressure is
                        # moot.  Use two independent 1-bank sacc tiles and
                        # interleave recip/mul per chunk so the tile-granular
                        # WAR hazard (fold_c1 vs recip_c0 on a shared tile)
                        # can't serialize the tail.
                        saccs = [
                            psc.tile(
                                [128, 512], F32, tag="sc", name=f"sacc{ci}_{half}"
                            )
                            for ci in range(2)
                        ]
                        for ci, c in enumerate(range(0, S_HALF, 512)):
                            nc.tensor.matmul(
                                saccs[ci][:],
                                ones_h[:],
                                st["eacc"][:, c : c + 512],
                                start=True,
                                stop=True,
                            )
                        for ci, c in enumerate(range(0, S_HALF, 512)):
                            rec = outs.tile([128, 512], F32, tag=f"rec{ci}")
                            nc.vector.reciprocal_approx_fast(
                                out=rec[:], in_=saccs[ci][:]
                            )
                            osb = outs.tile([128, 512], F32, tag=f"osb{ci}")
                            nc.vector.tensor_mul(osb[:], st["oacc"][ci][:], rec[:])
                            nc.sync.dma_start(
                                out=ot[pair, :, s0 + c : s0 + c + 512], in_=osb[:]
                            )
                        del half_state[half]
                        return
                    sacc = psc.tile([128, S_HALF], F32, tag="sc", name=f"sacc_{half}")
                    recs = []
                    # both folds, then both recips, then the muls: WAR
                    # tracking is tile-granular, so a recip emitted between
                    # the folds would serialize fold_c1 behind it.  The muls
                    # free the oacc banks for the next half's PV.
                    for ci, c in enumerate(range(0, S_HALF, 512)):
                        nc.tensor.matmul(
                            sacc[:, c : c + 512],
                            ones_h[:],
                            st["eacc"][:, c : c + 512],
                            start=True,
                            stop=True,
                        )
                    for ci, c in enumerate(range(0, S_HALF, 512)):
                        rec = outs.tile([128, 512], F32, tag=f"rec{ci}")
                        nc.vector.reciprocal_approx_fast(
                            out=rec[:], in_=sacc[:, c : c + 512]
                        )
                        recs.append(rec)
                    for ci, c in enumerate(range(0, S_HALF, 512)):
                        osb = outs.tile([128, 512], F32, tag=f"osb{ci}")
                        nc.vector.tensor_mul(osb[:], st["oacc"][ci][:], recs[ci][:])
                        nc.sync.dma_start(
                            out=ot[pair, :, s0 + c : s0 + c + 512], in_=osb[:]
                        )
                    del half_state[half]

            # ---- the flat pipeline ----
            for w in range(LOOKAHEAD):
                qk(w)
            for w in range(N_ITEMS):
                if w + LOOKAHEAD < N_ITEMS:
                    qk(w + LOOKAHEAD)
                exp(w)
                if w >= 1:
                    consume(w - 1)
            consume(N_ITEMS - 1)

    nc.finalize()
    return nc


def _get_program():
    if "nc" not in _cache:
        _cache["nc"] = _build_program()
    return _cache["nc"]


def make_in_maps(q: np.ndarray, k: np.ndarray, v: np.ndarray) -> list:
    q4 = np.ascontiguousarray(q, dtype=np.float32).reshape(BH, S, D)
    k4 = np.ascontiguousarray(k, dtype=np.float32).reshape(BH, S, D)
    v4 = np.ascontiguousarray(v, dtype=np.float32).reshape(BH, S, D)

    in_maps = []
    for core in range(N_CORES):
        sl = slice(core * BH_PER_CORE, (core + 1) * BH_PER_CORE)
        in_maps.append(
            {
                "qt": np.ascontiguousarray(
                    q4[sl].transpose(0, 2, 1)
                ).astype(np.float16),
                "kt": np.ascontiguousarray(
                    k4[sl].transpose(0, 2, 1)
                ).astype(np.float16),
                # [i, t*128+p, d] -> [i, p, t*128+d]
                "v": np.ascontiguousarray(
                    v4[sl]
                    .reshape(BH_PER_CORE, T_TILES, 128, D)
                    .transpose(0, 2, 1, 3)
                    .reshape(BH_PER_CORE, 128, T_TILES * D)
                ).astype(np.float16),
            }
        )
    return in_maps


def kernel(q: np.ndarray, k: np.ndarray, v: np.ndarray) -> np.ndarray:
    from concourse.bass_utils import run_bass_kernel_spmd

    nc = _get_program()
    in_maps = make_in_maps(q, k, v)
    res = run_bass_kernel_spmd(nc, in_maps, core_ids=list(range(N_CORES)))

    out = np.empty((BH, S, D), dtype=np.float32)
    for core in range(N_CORES):
        ot = res.results[core]["ot"]  # [BH_PER_CORE, D, S]
        out[core * BH_PER_CORE : (core + 1) * BH_PER_CORE] = ot.transpose(0, 2, 1)
    return out.reshape(B, H, S, D)


# revision 14
# speedup vs baseline: 1.0119x; 1.0119x over previous
"""Trainium2 Bass kernel for multi-head attention.

Problem: B=4, H=16, S=2048, D=128, fp32.
  scores = (q @ k^T) / sqrt(128); probs = softmax(scores, -1); out = probs @ v

Sharding: 64 (b,h) pairs -> 8 cores x 8 pairs. Fully independent per pair.

Per-(b,h) layout on device (T-layout, contraction dims on partitions):
  qT, kT: [D=128, S=2048] fp16 in SBUF (host pre-transposes + casts).
  For each t-tile (128 keys):
    scoresT[t, s] = kT[:, t-tile].T @ qT      (PE fp16, fp32 psum)
    expT = exp(scoresT / sqrt(D))             (ACT, fused scale, fp16 out)
    outT[d, s]  += v_tile[t, :].T @ expT      (PE fp16, accumulate in psum)
    eacc += expT                              (DVE fp16, 2x perf mode)
  denom = ones.T @ eacc (PE), rec = 1/denom (DVE), out = outT * rec (DVE).

The whole kernel is ONE flat software pipeline over work items
w = (pair, half, t): QK matmuls run 2 items ahead of the exp stream and
cross half/pair boundaries, so the ACT engine (the roofline: S^2/128
lanes @ 1.2 GHz ~= 266 us/core) never starves.  The normalize tail of
each half is chunked in 512-wide halves (oacc lives in 2 single-bank
psum tiles) so the next half's PV accumulation only waits for the chunk
it replaces.

PSUM: scores pool 3 slots x 2 banks (sacc shares the pool slots at each
half's tail) + 2 x 1-bank outT accumulators = 8 banks exactly.

fp16 everywhere off the psum path: q/k/v inputs, exp outputs, denominator
accumulator.  End-to-end L2 error vs the fp32 reference ~3e-4.
"""

import sys

sys.path.insert(0, "/opt/trn_rl_repo")

import numpy as np

B, H, S, D = 4, 16, 2048, 128
N_CORES = 8
BH = B * H                      # 64 pairs
BH_PER_CORE = BH // N_CORES     # 8
T_TILES = S // 128              # 16
S_HALF = S // 2                 # 1024
N_HALVES = BH_PER_CORE * 2      # 16 halves per core
N_ITEMS = N_HALVES * T_TILES    # 256 work items per core
SCALE = float(D) ** -0.5
LOOKAHEAD = 2                   # QK runs this many items ahead of exp

_cache = {}


def _build_program():
    import concourse.tile as tile
    from concourse import bacc, mybir

    F32 = mybir.dt.float32
    F16 = mybir.dt.float16

    nc = bacc.Bacc("TRN2", target_bir_lowering=False, debug=False)

    qt = nc.dram_tensor("qt", [BH_PER_CORE, D, S], F16, kind="ExternalInput")
    kt = nc.dram_tensor("kt", [BH_PER_CORE, D, S], F16, kind="ExternalInput")
    # v pre-shuffled on host to [p, t, d] so the load is fully contiguous
    v = nc.dram_tensor("v", [BH_PER_CORE, 128, T_TILES * D], F16, kind="ExternalInput")
    ot = nc.dram_tensor("ot", [BH_PER_CORE, D, S], F32, kind="ExternalOutput")

    with tile.TileContext(nc) as tc:
        with (
            tc.tile_pool(name="const", bufs=1) as const,
            tc.tile_pool(name="rin", bufs=2) as rin,
            tc.tile_pool(name="exps", bufs=8) as exps,
            tc.tile_pool(name="accp", bufs=3) as accp,
            tc.tile_pool(name="outs", bufs=6) as outs,
            tc.tile_pool(name="psc", bufs=3, space="PSUM") as psc,
            tc.tile_pool(name="pacc", bufs=1, space="PSUM") as pacc,
        ):
            ones_f = const.tile([128, 128], F32)
            nc.vector.memset(ones_f[:], 1.0)
            ones_h = const.tile([128, 128], F16)
            nc.vector.tensor_copy(ones_h[:], ones_f[:])

            # PE warm-up: dummy matmuls while the first input DMAs are in
            # flight, so the tensor engine's p-state is ramped (0.65 -> 2.4
            # GHz) by the time real work arrives.  Output is never read; the
            # first real scores matmul overwrites the slot with start=True.
            warm = psc.tile([128, S_HALF], F32, tag="sc", name="warm")
            for wi in range(8):
                nc.tensor.matmul(
                    warm[:, :128], ones_h[:], ones_h[:], start=True, stop=True
                )

            # per-pair input tiles, managed by the flat pipeline
            rin_tiles = {}   # pair -> (q_r, k_r, v_r)

            def load_pair(i):
                q_r = rin.tile([D, S], F16, tag="q_r")
                k_r = rin.tile([D, S], F16, tag="k_r")
                v_r = rin.tile([128, T_TILES, D], F16, tag="v_r")
                # order: what the first tiles need comes first.  All 16
                # k-tiles and v-tiles are consumed within the FIRST half, so
                # k gets priority over the second half of q.
                v_in = v[i].rearrange("p (t d) -> p t d", t=T_TILES)
                nc.sync.dma_start(out=k_r[:, :128], in_=kt[i, :, :128])
                nc.sync.dma_start(out=q_r[:, :S_HALF], in_=qt[i, :, :S_HALF])
                nc.sync.dma_start(out=k_r[:, 128:S_HALF], in_=kt[i, :, 128:S_HALF])
                nc.sync.dma_start(out=v_r[:, :4], in_=v_in[:, :4])
                nc.sync.dma_start(out=k_r[:, S_HALF:], in_=kt[i, :, S_HALF:])
                nc.sync.dma_start(out=v_r[:, 4:], in_=v_in[:, 4:])
                nc.sync.dma_start(out=q_r[:, S_HALF:], in_=qt[i, :, S_HALF:])
                rin_tiles[i] = (q_r, k_r, v_r)

            # per-half state, filled lazily by the pipeline
            half_state = {}  # half index -> dict

            def qk(w):
                """Emit the scores matmuls for work item w (PE)."""
                half, t = divmod(w, T_TILES)
                pair, h = divmod(half, 2)
                if t == 0 and h == 0:
                    load_pair(pair)
                q_r, k_r, _ = rin_tiles[pair]
                s0 = h * S_HALF
                sc = psc.tile([128, S_HALF], F32, tag="sc", name=f"sc_{w}")
                for c in range(0, S_HALF, 512):
                    nc.tensor.matmul(
                        sc[:, c : c + 512],
                        k_r[:, t * 128 : (t + 1) * 128],
                        q_r[:, s0 + c : s0 + c + 512],
                        start=True,
                        stop=True,
                    )
                st = half_state.setdefault(
                    half, {"sc": {}, "ets": {}, "oacc": None, "eacc": None}
                )
                st["sc"][t] = sc

            def exp(w):
                """Emit the exp for work item w (ACT)."""
                half, t = divmod(w, T_TILES)
                st = half_state[half]
                et = exps.tile([128, S_HALF], F16, tag="et", name=f"et_{w}")
                nc.scalar.activation(
                    et[:],
                    st["sc"].pop(t)[:],
                    mybir.ActivationFunctionType.Exp,
                    scale=SCALE,
                )
                st["ets"][t] = et

            def consume(w):
                """Emit PV + denominator accumulation for item w; at the end
                of a half, the fold/recip/mul/store tail (chunked)."""
                half, t = divmod(w, T_TILES)
                pair, h = divmod(half, 2)
                _, _, v_r = rin_tiles[pair]
                st = half_state[half]
                if t == 0:
                    st["oacc"] = [
                        pacc.tile(
                            [128, 512], F32, tag=f"oacc{ci}", name=f"oacc{ci}_{half}"
                        )
                        for ci in range(2)
                    ]
                et = st["ets"][t]
                for ci, c in enumerate(range(0, S_HALF, 512)):
                    nc.tensor.matmul(
                        st["oacc"][ci][:],
                        v_r[:, t, :],
                        et[:, c : c + 512],
                        start=(t == 0),
                        stop=(t == T_TILES - 1),
                    )
                if t == 1:
                    st["eacc"] = accp.tile(
                        [128, S_HALF], F16, tag="eacc", name=f"eacc_{half}"
                    )
                    nc.vector.tensor_add(
                        st["eacc"][:], st["ets"].pop(0)[:], st["ets"].pop(1)[:]
                    )
                elif t > 1:
                    nc.vector.tensor_add(
                        st["eacc"][:], st["eacc"][:], st["ets"].pop(t)[:]
                    )
                if t == T_TILES - 1:
                    s0 = h * S_HALF
                    if half == N_HALVES - 1:
                        # Final half: nothing follows, so psum pressure is
                        # moot.  Use two independent 1-bank sacc tiles and
                        # interleave recip/mul per chunk so the tile-granular
                        # WAR hazard (fold_c1 vs recip_c0 on a shared tile)
                        # can't serialize the tail.
                        saccs = [
                            psc.tile(
                                [128, 512], F32, tag="sc", name=f"sacc{ci}_{half}"
                            )
                            for ci in range(2)
                        ]
                        for ci, c in enumerate(range(0, S_HALF, 512)):
                            nc.tensor.matmul(
                                saccs[ci][:],
                                ones_h[:],
                                st["eacc"][:, c : c + 512],
                                start=True,
                                stop=True,
                            )
                        for ci, c in enumerate(range(0, S_HALF, 512)):
                            rec = outs.tile([128, 512], F32, tag=f"rec{ci}")
                            nc.vector.reciprocal_approx_fast(
                                out=rec[:], in_=saccs[ci][:]
                            )
                            osb = outs.tile([128, 512], F32, tag=f"osb{ci}")
                            nc.vector.tensor_mul(osb[:], st["oacc"][ci][:], rec[:])
                            nc.sync.dma_start(
                                out=ot[pair, :, s0 + c : s0 + c + 512], in_=osb[:]
                            )
                        del half_state[half]
                        return
                    sacc = psc.tile([128, S_HALF], F32, tag="sc", name=f"sacc_{half}")
                    recs = []
                    # both folds, then both recips, then the muls: WAR
                    # tracking is tile-granular, so a recip emitted between
                    # the folds would serialize fold_c1 behind it.  The muls
                    # free the oacc banks for the next half's PV.
                    for ci, c in enumerate(range(0, S_HALF, 512)):
                        nc.tensor.matmul(
                            sacc[:, c : c + 512],
                            ones_h[:],
                            st["eacc"][:, c : c + 512],
                            start=True,
                            stop=True,
                        )
                    for ci, c in enumerate(range(0, S_HALF, 512)):
                        rec = outs.tile([128, 512], F32, tag=f"rec{ci}")
                        nc.vector.reciprocal_approx_fast(
                            out=rec[:], in_=sacc[:, c : c + 512]
                        )
                        recs.append(rec)
                    for ci, c in enumerate(range(0, S_HALF, 512)):
                        osb = outs.tile([128, 512], F32, tag=f"osb{ci}")
                        nc.vector.tensor_mul(osb[:], st["oacc"][ci][:], recs[ci][:])
                        nc.sync.dma_start(
                            out=ot[pair, :, s0 + c : s0 + c + 512], in_=osb[:]
                        )
                    del half_state[half]

            # ---- the flat pipeline ----
            for w in range(LOOKAHEAD):
                qk(w)
            for w in range(N_ITEMS):
                if w + LOOKAHEAD < N_ITEMS:
                    qk(w + LOOKAHEAD)
                exp(w)
                if w >= 1:
                    consume(w - 1)
            consume(N_ITEMS - 1)

    nc.finalize()
    return nc


def _get_program():
    if "nc" not in _cache:
        _cache["nc"] = _build_program()
    return _cache["nc"]


def make_in_maps(q: np.ndarray, k: np.ndarray, v: np.ndarray) -> list:
    q4 = np.ascontiguousarray(q, dtype=np.float32).reshape(BH, S, D)
    k4 = np.ascontiguousarray(k, dtype=np.float32).reshape(BH, S, D)
    v4 = np.ascontiguousarray(v, dtype=np.float32).reshape(BH, S, D)

    in_maps = []
    for core in range(N_CORES):
        sl = slice(core * BH_PER_CORE, (core + 1) * BH_PER_CORE)
        in_maps.append(
            {
                "qt": np.ascontiguousarray(
                    q4[sl].transpose(0, 2, 1)
                ).astype(np.float16),
                "kt": np.ascontiguousarray(
                    k4[sl].transpose(0, 2, 1)
                ).astype(np.float16),
                # [i, t*128+p, d] -> [i, p, t*128+d]
                "v": np.ascontiguousarray(
                    v4[sl]
                    .reshape(BH_PER_CORE, T_TILES, 128, D)
                    .transpose(0, 2, 1, 3)
                    .reshape(BH_PER_CORE, 128, T_TILES * D)
                ).astype(np.float16),
            }
        )
    return in_maps


def kernel(q: np.ndarray, k: np.ndarray, v: np.ndarray) -> np.ndarray:
    from concourse.bass_utils import run_bass_kernel_spmd

    nc = _get_program()
    in_maps = make_in_maps(q, k, v)
    res = run_bass_kernel_spmd(nc, in_maps, core_ids=list(range(N_CORES)))

    out = np.empty((BH, S, D), dtype=np.float32)
    for core in range(N_CORES):
        ot = res.results[core]["ot"]  # [BH_PER_CORE, D, S]
        out[core * BH_PER_CORE : (core + 1) * BH_PER_CORE] = ot.transpose(0, 2, 1)
    return out.reshape(B, H, S, D)
